# revision 21
# baseline (speedup 1.0000x reference)
"""Multi-head attention (B=4, T=2048, E=1024, H=16, D=64) on 8 TRN2 cores.

Sharding: core c handles batch b = c//2 and heads hg = c%2 (8 heads each).
No cross-device comms: each core emits a partial out-projection over its
512 head-columns in bf16; the host upcasts and sums the two partials per
batch (the tensor-parallel all-reduce).

Design (all operands bf16, fp32 PSUM accumulation):
  - Projection block P(tb) and attention block A(tb-1) are co-emitted in
    windows: P matmul chains are chopped into credit-paced "filler" steps
    interleaved between attention QK->exp->AV slots so PE never idles
    while ACT computes the exp.
  - Attention runs two heads per pipeline in alternating slots - each
    head's QK/AV hides the other head's exp latency with only two score
    tiles of PSUM (8-bank budget: 2 proj/out-proj accumulators + 2 o
    accumulators + 2x2-bank score pairs).
  - Scores land in [128, 2, TB] pair tiles; off-diagonal exps are fused
    1024-wide; causal zeroing (gpsimd affine_select) covers only the
    128-wide block where the boundary passes. bf16 lets the diagonal
    QK/AV matmuls shrink to their live width (bf16 runs 1 cycle/row at
    any width; f32r needs >= 256).
  - RoPE rotate-half is a single DVE stream_shuffle: the head dim is
    pre-permuted host-side so each rotation partner sits 16 partitions
    away within a 32-partition quadrant, and the sign is folded into the
    sin table. No PE/PSUM involvement.
  - Out-projections for q-blocks 0-2 are deferred into the final
    (ACT-bound) attention window as PE filler; q-block 3's chains run at
    the tail with ACT/DVE-alternating PSUM evacuation so slot release
    never queues behind the final softmax divides.
  - One dma_start per tensor/block (HWDGE is serialized at ~650ns per
    descriptor batch); the first wk/x chunks stream in quarters so the
    first projection chain starts ~3.5us in; y is stored bf16.
"""
import sys
import numpy as np
from collections import deque
from contextlib import ExitStack

try:
    import concourse  # noqa: F401
except ImportError:
    sys.path.insert(0, "/opt/trn_rl_repo")

import ml_dtypes  # noqa: E402
import concourse.tile as tile  # noqa: E402
from concourse import bacc, mybir  # noqa: E402
from concourse.bass_utils import run_bass_kernel_spmd  # noqa: E402

F32 = mybir.dt.float32
BF16 = mybir.dt.bfloat16
AF = mybir.ActivationFunctionType

B, T, E, H, D = 4, 2048, 1024, 16, 64
N_CORES = 8
HPC = 8            # heads per core
EC = HPC * D       # 512 head-columns per core
TB = 512           # t/q block
KC = 128           # k chunk
NTB = T // TB      # 4
NTT = T // KC      # 16
CCH = E // 128     # 8 contraction chunks for x projections
OCH = EC // 128    # 4 chunks of the per-core head-column dim
ROPE_BASE = 10000.0

_NC = None


class _Filler:
    """Queue of deferred emission steps with PE-ns cost weights:
    generators advance one yield per step, callables run once. run_ns()
    spends a PE-time budget so filler coverage spreads evenly instead of
    exhausting early."""

    def __init__(self):
        self.items = deque()

    def add_gen(self, gen, step_ns):
        self.items.append(("g", gen, None, step_ns))

    def add_call(self, fn, a, cost_ns):
        self.items.append(("c", fn, a, cost_ns))

    balance = 0.0

    def step(self):
        """Returns the PE-ns cost of the emitted step, or 0 if empty."""
        while self.items:
            kind, obj, a, cost = self.items[0]
            if kind == "g":
                try:
                    next(obj)
                    return cost
                except StopIteration:
                    self.items.popleft()
                    continue
            self.items.popleft()
            obj(*a)
            return cost
        return 0

    def run_ns(self, budget):
        """Credit `budget` PE-ns and emit items while in credit. Overdraft
        carries so coarse items don't starve later call sites."""
        self.balance += budget
        while self.balance > 0:
            c = self.step()
            if c == 0:
                self.balance = 0
                return
            self.balance -= c

    def drain(self):
        while self.step():
            pass


def _build():
    nc = bacc.Bacc("TRN2", target_bir_lowering=False, debug=False,
                   num_devices=N_CORES)
    ap = {}
    def din(name, shape, dt=BF16):
        ap[name] = nc.dram_tensor(name, shape, dt, kind="ExternalInput").ap()
    din("xT", [E, T])              # x[b].T
    din("wqT", [E, EC])            # Wq[cols,:].T
    din("wkT", [E, EC])
    din("wvT", [E, EC])
    din("woT", [EC, E])            # Wo[:,cols].T
    din("cosb", [128, T], F32)     # cos dup'd over 2 heads, [2*64, T]
    din("sinb", [128, T], F32)
    y = nc.dram_tensor("y", [T, E], BF16, kind="ExternalOutput").ap()

    with tile.TileContext(nc) as tc, ExitStack() as ctx:
        persist = ctx.enter_context(tc.tile_pool(name="persist", bufs=1))
        qT = persist.tile([128, OCH, T], BF16, tag="qT")
        kT = persist.tile([128, OCH, T], BF16, tag="kT")
        vv = persist.tile([128, NTT, HPC, D + 1], BF16, tag="vv")
        oT = persist.tile([128, OCH, T], BF16, tag="oT")
        wq_sb = persist.tile([128, CCH, EC], BF16, tag="wq")
        wk_sb = persist.tile([128, CCH, EC], BF16, tag="wk")
        wv_sb = persist.tile([128, CCH, EC], BF16, tag="wv")
        wo_sb = persist.tile([128, OCH, E], BF16, tag="wo")
        ones_sb = persist.tile([128, HPC], BF16, tag="ones")
        nc.vector.memset(ones_sb[:], 1.0)

        xt_pool = ctx.enter_context(tc.tile_pool(name="xt", bufs=2))
        cs_pool = ctx.enter_context(tc.tile_pool(name="cs", bufs=2))
        tmp_pool = ctx.enter_context(tc.tile_pool(name="tmp", bufs=2))
        e_pool = ctx.enter_context(tc.tile_pool(name="e", bufs=6))
        r_pool = ctx.enter_context(tc.tile_pool(name="r", bufs=4))
        b_pool = ctx.enter_context(tc.tile_pool(name="b", bufs=4))
        ysb_pool = ctx.enter_context(tc.tile_pool(name="ysb", bufs=6))
        # PSUM budget (8 banks): acc 2 + o/sw 2 + s 2x2 = 8
        acc_pool = ctx.enter_context(
            tc.tile_pool(name="acc", bufs=2, space="PSUM"))
        os_pool = ctx.enter_context(
            tc.tile_pool(name="os", bufs=2, space="PSUM"))
        s_pool = ctx.enter_context(
            tc.tile_pool(name="s", bufs=2, space="PSUM"))

        xTr = ap["xT"].rearrange("(c p) t -> p c t", p=128)

        # initial DMAs: wk/xt0 interleaved in quarters so the first k chain
        # starts ~3us and chases chunk arrivals; everything else
        # whole-tensor (one HWDGE slot each)
        wk_src = ap["wkT"].rearrange("(c p) e -> p c e", p=128)
        xt0 = xt_pool.tile([128, CCH, TB], BF16, tag="xt")
        qc = CCH // 4
        for q_ in range(4):
            cs_ = slice(q_ * qc, (q_ + 1) * qc)
            nc.sync.dma_start(out=wk_sb[:, cs_, :], in_=wk_src[:, cs_, :])
            nc.sync.dma_start(out=xt0[:, cs_, :], in_=xTr[:, cs_, 0:TB])
        cos0 = cs_pool.tile([128, TB], F32, tag="cos")
        sin0 = cs_pool.tile([128, TB], F32, tag="sin")
        nc.sync.dma_start(out=cos0, in_=ap["cosb"][:, 0:TB])
        nc.sync.dma_start(out=sin0, in_=ap["sinb"][:, 0:TB])
        nc.sync.dma_start(
            out=wq_sb, in_=ap["wqT"].rearrange("(c p) e -> p c e", p=128))
        nc.sync.dma_start(
            out=wv_sb, in_=ap["wvT"].rearrange("(c p) e -> p c e", p=128))
        nc.sync.dma_start(
            out=wo_sb, in_=ap["woT"].rearrange("(c p) e -> p c e", p=128))

        def dma_block(tb):
            ts = slice(tb * TB, (tb + 1) * TB)
            xt = xt_pool.tile([128, CCH, TB], BF16, tag="xt")
            nc.sync.dma_start(out=xt[:], in_=xTr[:, :, ts])
            cos_sb = cs_pool.tile([128, TB], F32, tag="cos")
            sin_sb = cs_pool.tile([128, TB], F32, tag="sin")
            nc.sync.dma_start(out=cos_sb, in_=ap["cosb"][:, ts])
            nc.sync.dma_start(out=sin_sb, in_=ap["sinb"][:, ts])
            return xt, cos_sb, sin_sb

        def p_work(tb, xt, cos_sb, sin_sb):
            """P(tb): k/q/v projections + RoPE for t-block tb; every yield
            boundary carries ~2 matmuls of PE work."""
            ts = slice(tb * TB, (tb + 1) * TB)
            pending_rope = []

            def emit_rope(dst, m):
                qs = tmp_pool.tile([128, TB], BF16, tag="qs")
                nc.vector.stream_shuffle(
                    qs[:], dst[:, m, ts],
                    mask=list(range(16, 32)) + list(range(0, 16)))
                t1 = tmp_pool.tile([128, TB], F32, tag="t1")
                nc.vector.tensor_mul(t1[:], dst[:, m, ts], cos_sb[:])
                t2 = tmp_pool.tile([128, TB], F32, tag="t2")
                nc.vector.tensor_mul(t2[:], qs[:], sin_sb[:])
                nc.vector.tensor_add(dst[:, m, ts], t1[:], t2[:])

            # K first so attention on later windows never waits on it
            for w_sb, dst in ((wk_sb, kT), (wq_sb, qT)):
                for m in range(OCH):
                    pp = acc_pool.tile([128, TB], F32, tag="acc")
                    for c in range(CCH):
                        nc.tensor.matmul(
                            pp[:], w_sb[:, c, m * 128:(m + 1) * 128],
                            xt[:, c, :], start=(c == 0), stop=(c == CCH - 1))
                        if c in (1, 3, 5):
                            yield
                    nc.scalar.activation(dst[:, m, ts], pp[:], AF.Copy)
                    # RoPE for chain m deferred behind chain m+1's matmuls
                    # so the swap matmul never stalls PE on the evacuation
                    pending_rope.append((dst, m))
                    if len(pending_rope) > 1:
                        emit_rope(*pending_rope.pop(0))
                    yield
            for st in range(TB // KC):
                tt = tb * (TB // KC) + st
                pp = acc_pool.tile([128, EC], F32, tag="acc")
                for c in range(CCH):
                    nc.tensor.matmul(
                        pp[:], xt[:, c, st * 128:(st + 1) * 128],
                        wv_sb[:, c, :], start=(c == 0), stop=(c == CCH - 1))
                    if c in (1, 3, 5):
                        yield
                nc.scalar.activation(
                    vv[:, tt, :, 0:D],
                    pp[:].rearrange("p (h d) -> p h d", d=D), AF.Copy)
                nc.vector.tensor_copy(vv[:, tt, :, D], ones_sb[:])
                yield
            for args in pending_rope:
                emit_rope(*args)

        # ---------------- attention pieces ----------------
        def qk_pair(qb, h, g):
            po = (h % 2) * 64
            ch = h // 2
            s_ps = s_pool.tile([128, 2, TB], F32, tag="s")
            for i in (0, 1):
                kc = 2 * g + i
                j = kc - 4 * qb
                co = 0 if j <= 0 else 128 * j
                nc.tensor.matmul(
                    s_ps[:, i, co:],
                    kT[po:po + D, ch, kc * KC:(kc + 1) * KC],
                    qT[po:po + D, ch, qb * TB + co:(qb + 1) * TB],
                    start=True, stop=True)
            return s_ps

        def exp_pair(qb, g, s_ps):
            e_sb = e_pool.tile([128, 2, TB], BF16, tag="e")
            if 2 * g + 1 < 4 * qb:
                # fully off-diagonal pair: one fused 1024-wide exp
                nc.scalar.activation(e_sb[:], s_ps[:], AF.Exp, scale=0.125)
            else:
                for i in (0, 1):
                    kc = 2 * g + i
                    j = kc - 4 * qb
                    co = 0 if j <= 0 else 128 * j
                    nc.scalar.activation(
                        e_sb[:, i, co:], s_ps[:, i, co:], AF.Exp, scale=0.125)
                    # zero the triangle inside the 128-wide block where the
                    # causal boundary passes; cols left of co are never read
                    nc.gpsimd.affine_select(
                        out=e_sb[:, i, co:co + KC],
                        in_=e_sb[:, i, co:co + KC],
                        compare_op=mybir.AluOpType.is_ge,
                        fill=0.0, base=0, pattern=[[1, KC]],
                        channel_multiplier=-1)
            return e_sb

        def av_pair(qb, h, o_ps, g, e_sb):
            nkv = (qb + 1) * 4
            for i in (0, 1):
                kc = 2 * g + i
                j = kc - 4 * qb
                co = 0 if j <= 0 else 128 * j
                nc.tensor.matmul(
                    o_ps[:, co:], vv[:, kc, h, :], e_sb[:, i, co:],
                    start=(kc == 0), stop=(kc == nkv - 1))

        def divide(qb, h, o_ps, split=1):
            po = (h % 2) * 64
            ch = h // 2
            r_sb = r_pool.tile([1, TB], F32, tag="r")
            nc.vector.reciprocal(r_sb[:], o_ps[D:D + 1, :])
            rb_sb = b_pool.tile([D, TB], F32, tag="rb")
            nc.gpsimd.partition_broadcast(rb_sb[:], r_sb[:])
            w = TB // split
            for p_ in range(split):
                cs_ = slice(p_ * w, (p_ + 1) * w)
                nc.vector.tensor_mul(
                    oT[po:po + D, ch, qb * TB + p_ * w:qb * TB + (p_ + 1) * w],
                    o_ps[0:D, cs_], rb_sb[:, cs_])

        def emit_attention(qb, fill, slot_ns, boundary_ns, init_ns=500):
            """Two heads in an alternating-slot pipeline: each head's QK/AV
            covers the other head's exp latency."""
            npair = (qb + 1) * 2
            for hh in (0, 2, 4, 6):
                heads = (hh, hh + 1)
                o_ps = {h: os_pool.tile([D + 1, TB], F32, tag="os",
                                        name=f"o{h}")
                        for h in heads}
                s_cur = {}
                s_cur[hh] = qk_pair(qb, hh, 0)
                fill.run_ns(init_ns)
                s_cur[hh + 1] = qk_pair(qb, hh + 1, 0)
                fill.run_ns(init_ns)
                for g in range(npair):
                    for h in heads:
                        e_cur = exp_pair(qb, g, s_cur[h])
                        if g + 1 < npair:
                            s_cur[h] = qk_pair(qb, h, g + 1)
                        fill.run_ns(slot_ns)
                        av_pair(qb, h, o_ps[h], g, e_cur)
                # boosted priority: the o-slot release gates the next
                # head-pair's AV; this orders the divides ahead of
                # later-emitted rope work in the DVE/Pool queues
                with tc.high_priority(offset=60):
                    for h in heads:
                        divide(qb, h, o_ps[h])
                fill.run_ns(boundary_ns)

        def emit_yproj(qb, st, eh, evac_act=False):
            tt = qb * (TB // KC) + st
            tsl = slice(tt * 128, (tt + 1) * 128)
            y_ps = acc_pool.tile([128, 512], F32, tag="acc")
            for c in range(OCH):
                nc.tensor.matmul(
                    y_ps[:], oT[:, c, tsl],
                    wo_sb[:, c, eh * 512:(eh + 1) * 512],
                    start=(c == 0), stop=(c == OCH - 1))
            y_sb = ysb_pool.tile([128, 512], BF16, tag="ysb")
            if evac_act:
                # tail chains: ACT is idle there and, unlike DVE, its queue
                # is not behind the final divides — PSUM slots free sooner
                nc.scalar.activation(y_sb[:], y_ps[:], AF.Copy)
            else:
                nc.vector.tensor_copy(y_sb[:], y_ps[:])
            nc.sync.dma_start(
                out=y[tsl, eh * 512:(eh + 1) * 512], in_=y_sb[:])

        # ---------------- window loop ----------------
        nxt = (xt0, cos0, sin0)
        per_w = {0: (600, 2600), 1: (400, 2000), 2: (230, 1800)}
        for tb in range(NTB):
            xt, cos_sb, sin_sb = nxt
            if tb + 1 < NTB:
                nxt = dma_block(tb + 1)
            fill = _Filler()
            fill.add_gen(p_work(tb, xt, cos_sb, sin_sb), 500)
            if tb == 0:
                fill.drain()
            else:
                sl_, bd_ = per_w[tb - 1]
                emit_attention(tb - 1, fill, sl_, bd_)
                fill.drain()
        # final window: A(3) + all deferred out-projections as filler
        fill = _Filler()
        n_ = 0
        for qb in range(NTB - 1):
            for st in range(TB // KC):
                for eh in range(2):
                    # the last fillers run next to the final divides: ACT
                    # evacuation keeps their PSUM release off the DVE queue
                    fill.add_call(emit_yproj, (qb, st, eh, n_ >= 20), 853)
                    n_ += 1
        emit_attention(NTB - 1, fill, 0, 2000)
        fill.drain()
        for st in range(TB // KC):
            for eh in range(2):
                emit_yproj(3, st, eh, evac_act=(eh == 0))
    nc.compile()
    return nc


def _host_inputs(x, Wq, Wk, Wv, Wo):
    # rope tables in [e, t] layout, duplicated across the 2 heads of a chunk
    inv_freq = 1.0 / (ROPE_BASE ** (np.arange(0, D, 2, dtype=np.float64) / D))
    freqs = np.outer(np.arange(T, dtype=np.float64), inv_freq)  # [T, 32]
    emb = np.concatenate([freqs, freqs], axis=-1)               # [T, 64]
    cos1, sin1 = np.cos(emb).T, np.sin(emb).T                   # [64, T]
    cosb = np.concatenate([cos1, cos1], 0).astype(np.float32)   # [128, T]
    sinb = np.concatenate([sin1, sin1], 0).astype(np.float32)

    # head-dim permutation: rope pairs (i, i+32) land 16 apart within a
    # 32-partition quadrant so rotate-half is one DVE stream_shuffle.
    # Scores are invariant as long as q and k share the permutation.
    perm = np.concatenate([np.arange(0, 16), np.arange(32, 48),
                           np.arange(16, 32), np.arange(48, 64)])
    cos1, sin1 = cos1[perm], sin1[perm]
    sgn = np.where(np.arange(64) % 32 < 16, -1.0, 1.0)[:, None]
    sin1 = sin1 * sgn
    cosb = np.concatenate([cos1, cos1], 0).astype(np.float32)
    sinb = np.concatenate([sin1, sin1], 0).astype(np.float32)
    pidx = np.concatenate([64 * g + perm for g in range(HPC)])

    def bf(a):
        return np.ascontiguousarray(a).astype(ml_dtypes.bfloat16)

    xTs = [bf(x[b].T) for b in range(B)]
    wmaps = []
    for hg in range(2):
        cols = slice(hg * EC, (hg + 1) * EC)
        wmaps.append({
            "wqT": bf(Wq[cols, :][pidx].T),
            "wkT": bf(Wk[cols, :][pidx].T),
            "wvT": bf(Wv[cols, :].T),
            "woT": bf(Wo[:, cols].T),
        })
    in_maps = []
    for c in range(N_CORES):
        b, hg = c // 2, c % 2
        in_maps.append({
            "xT": xTs[b], "cosb": cosb, "sinb": sinb,
            **wmaps[hg],
        })
    return in_maps


def kernel(x, causal_mask, Wq, Wk, Wv, Wo):
    global _NC
    x = np.asarray(x, dtype=np.float32)
    Wq = np.asarray(Wq, dtype=np.float32)
    Wk = np.asarray(Wk, dtype=np.float32)
    Wv = np.asarray(Wv, dtype=np.float32)
    Wo = np.asarray(Wo, dtype=np.float32)
    if _NC is None:
        _NC = _build()
    in_maps = _host_inputs(x, Wq, Wk, Wv, Wo)
    def _assemble(res):
        out = np.empty((B, T, E), dtype=np.float32)
        for b in range(B):
            out[b] = (res.results[2 * b]["y"].astype(np.float32)
                      + res.results[2 * b + 1]["y"].astype(np.float32))
        return out

    def _sane(out):
        # causal row 0 attends only to itself: y[b,0] == Wo @ (Wv @ x[b,0])
        # up to bf16 noise; plus global finiteness. Catches the (rare)
        # silent device corruption observed as whole-row NaNs.
        if not np.isfinite(out).all():
            return False
        for b in range(B):
            y0 = Wo @ (Wv @ x[b, 0])
            if np.abs(out[b, 0] - y0).max() > 0.05 * np.abs(y0).max() + 1e-3:
                return False
        return True

    out = None
    for attempt in range(4):
        try:
            res = run_bass_kernel_spmd(_NC, in_maps, list(range(N_CORES)))
            out = _assemble(res)
            if _sane(out):
                return out
        except Exception:
            # transient NRT/device hiccups recover on retry
            if attempt == 3:
                raise
        import time
        time.sleep(2 + 3 * attempt)
    return out
